# revision 6
# baseline (speedup 1.0000x reference)
"""AttentiveReduce Trainium2 kernel.

Reference computation (B=32, L=4096, D=768, H=8, Dh=96):
    xn   = LayerNorm(x; gamma1, beta1)            [B,L,D]
    kv   = xn @ w_kv.T ; k, v = split(kv)         [B,L,D] each
    dots = einsum('hd,blhd->bhl', q, k) * Dh^-0.5
    attn = softmax(dots, axis=-1)
    out  = einsum('bhl,blhd->bhd', attn, v) -> [B,D]
    out  = LayerNorm(out; gamma2, beta2)

Algebraic restructuring (exact up to fp rounding):
  - k only appears via q.k per head, so fold q into Wk on the host:
        qw[h,d] = Dh^-0.5 * sum_j q[h,j] * Wk[h*Dh+j, d]
        dots[b,h,l] = r_l*(x_l . (gamma1*qw_h)) - r_l*mu_l*s_h + c_h
    with LayerNorm stats mu_l, r_l = rsqrt(var_l+eps) and host scalars
    s_h = sum_d gamma1*qw_h, c_h = sum_d beta1*qw_h.
  - v is linear in xn, so pool x first and project after:
        P1[b,h,d] = sum_l u[b,h,l] x[b,l,d],  u = exp(dots)*r_l
        U[b,h] = sum_l u*mu_l,  Z[b,h] = sum_l u*sigma_l  (sigma=1/r)
        pooled = gamma1*(P1 - U)/Z + beta1 ; out = pooled @ Wv_h.T ; LN2
  - |dots| stays ~5 for this data, so softmax needs no max subtraction:
    one streaming pass over x.

Device computes P1/U/Z; the tiny epilogue runs on host over [32,8,768].

Device pipeline per batch (L=4096 = 8 macro tiles x 512 tokens):
  phase A (per macro): DMA x; PE-transpose x into d-partition layout;
    f32r logit matmul Y = [a_0..a_7, ones/D]^T @ x^T; transpose Y back to
    token-partition layout; per-token sum(x^2) split between ACT
    (Square+accum) and GPSIMD(mult)+DVE(accum).
  phase B (per batch): var -> r = exp(-0.5*ln(var+eps)) -> sigma, batched
    [128, 32] so the ACT exp/ln table sets load only twice per batch.
  phase C: batched u = exp(r*(y - mu*s) + c)*r over the whole batch, then
    per-p-tile f32r matmuls accumulate P1 = u^T @ [x | mu | sigma] in PSUM.

Sharding: data-parallel over batch: 8 cores x 4 batches, params replicated.
Per-core HBM traffic = 48MiB of x read once (memory-bound target).
fp32r (fp32 with 11-bit mantissa, 4x PE throughput) is used for the
matmuls; inputs are pre-rounded on host so HW truncation is exact RNE.
"""

import sys

if "/opt/trn_rl_repo" not in sys.path:
    sys.path.insert(0, "/opt/trn_rl_repo")

import numpy as np

import concourse.bacc as bacc
import concourse.tile as tile
from concourse import bass_utils, mybir

f32 = mybir.dt.float32
f32r = mybir.dt.float32r
AF = mybir.ActivationFunctionType
ALU = mybir.AluOpType

B, L, D, H, Dh = 32, 4096, 768, 8, 96
EPS = 1e-5
NCORES = 8
BPC = B // NCORES  # batches per core
PT = 128           # tokens per partition tile
MACRO = 512        # tokens per macro tile (4 p-tiles)
NPT = MACRO // PT  # 4
NC6 = D // 128     # 6 d-chunks of 128
OUTW = D + 2       # P1 row width: 768 x-cols + mu col + sigma col


def _build(bpc, nmac, use_c):
    """Per-core program: `bpc` batches x nmac*512 tokens each."""
    nc = bacc.Bacc("TRN2", target_bir_lowering=False, debug=False)

    x_in = nc.dram_tensor("x", [bpc, nmac * MACRO, D], f32r, kind="ExternalInput")
    g_in = nc.dram_tensor("gmat", [D, 9], f32r, kind="ExternalInput")
    sc_in = nc.dram_tensor("scvec", [128, 16], f32, kind="ExternalInput")
    id_in = nc.dram_tensor("ident", [128, 128], f32r, kind="ExternalInput")
    p1_out = nc.dram_tensor("p1out", [bpc, 8, OUTW], f32, kind="ExternalOutput")

    NW = nmac * NPT  # p-tiles per batch (32)

    with tile.TileContext(nc) as tc:
        with (
            tc.tile_pool(name="singles", bufs=1) as singles,
            tc.tile_pool(name="xe", bufs=nmac + 2) as xe_pool,
            tc.tile_pool(name="xt", bufs=2) as xt_pool,
            tc.tile_pool(name="ysb", bufs=2) as ysb_pool,
            tc.tile_pool(name="ytb", bufs=2) as ytb_pool,
            tc.tile_pool(name="uw", bufs=2) as uw_pool,
            tc.tile_pool(name="st", bufs=2) as st_pool,
            tc.tile_pool(name="junk", bufs=1) as junk_pool,
            tc.tile_pool(name="osb", bufs=2) as osb_pool,
            tc.tile_pool(name="ptp", bufs=4, space="PSUM") as ptp_pool,
            tc.tile_pool(name="yp", bufs=1, space="PSUM") as yp_pool,
            tc.tile_pool(name="ytp", bufs=1, space="PSUM") as ytp_pool,
            tc.tile_pool(name="p1p", bufs=1, space="PSUM") as p1p_pool,
        ):
            g_sb = singles.tile([128, NC6, 9], f32r)
            nc.sync.dma_start(out=g_sb, in_=g_in.rearrange("(c p) m -> p c m", p=128))
            sc_sb = singles.tile([128, 16], f32)
            nc.sync.dma_start(out=sc_sb, in_=sc_in[:, :])
            id_sb = singles.tile([128, 128], f32r)
            nc.sync.dma_start(out=id_sb, in_=id_in[:, :])
            eps_t = singles.tile([128, 1], f32)
            nc.vector.memset(eps_t, EPS)

            s_bc = (
                sc_sb[:, 0:8]
                .unsqueeze(1)
                .unsqueeze(1)
                .to_broadcast([128, nmac, NPT, 8])
            )
            c_bc = (
                sc_sb[:, 8:16]
                .unsqueeze(1)
                .unsqueeze(1)
                .to_broadcast([128, nmac, NPT, 8])
            )

            for b in range(bpc):
                # ---------------- phase A: stream macros ----------------
                xes = []
                ytb = ytb_pool.tile([128, nmac, NPT, 9], f32)
                ssq = st_pool.tile([128, NW], f32, tag="ssq")
                for m in range(nmac):
                    xe = xe_pool.tile([128, NPT, OUTW], f32r)
                    for hh in range(2):
                        src = x_in[
                            b, m * MACRO + hh * 256 : m * MACRO + (hh + 1) * 256, :
                        ].rearrange("(pt p) d -> p pt d", p=128)
                        nc.sync.dma_start(
                            out=xe[:, 2 * hh : 2 * hh + 2, 0:D], in_=src
                        )

                    xt = xt_pool.tile([128, NC6, MACRO], f32r)
                    for c in range(NC6):
                        xtp = ptp_pool.tile([128, MACRO], f32)
                        for pt in range(NPT):
                            nc.tensor.transpose(
                                xtp[:, pt * PT : (pt + 1) * PT].bitcast(f32r),
                                xe[:, pt, c * 128 : (c + 1) * 128],
                                id_sb[:, :],
                            )
                        nc.vector.tensor_copy(xt[:, c, :], xtp)

                    # Y rows 0-7 = x . a_h, row 8 = mu
                    yp = yp_pool.tile([9, MACRO], f32)
                    for c in range(NC6):
                        nc.tensor.matmul(
                            yp,
                            g_sb[:, c, :],
                            xt[:, c, :],
                            start=(c == 0),
                            stop=(c == NC6 - 1),
                        )
                    y_sb = ysb_pool.tile([9, MACRO], f32)
                    nc.vector.tensor_copy(y_sb, yp)
                    ytp = ytp_pool.tile([128, NPT, 9], f32)
                    for pt in range(NPT):
                        nc.tensor.transpose(
                            ytp[:, pt, :],
                            y_sb[:, pt * PT : (pt + 1) * PT],
                            id_sb[:9, :9].bitcast(f32),
                        )
                    nc.vector.tensor_copy(ytb[:, m, :, :], ytp)
                    # mu into the U column of x_ext
                    nc.vector.tensor_copy(xe[:, :, D : D + 1], ytp[:, :, 8:9])

                    # per-token sum(x^2): 2 p-tiles on ACT, 2 via GPSIMD+DVE
                    junk_a = junk_pool.tile([128, D], f32, tag="junk_a")
                    junk_g = junk_pool.tile([128, D], f32, tag="junk_g")
                    for pt in range(2):
                        nc.scalar.activation(
                            junk_a,
                            xe[:, pt, 0:D].bitcast(f32),
                            AF.Square,
                            accum_out=ssq[:, m * NPT + pt : m * NPT + pt + 1],
                        )
                    for pt in range(2, NPT):
                        nc.gpsimd.tensor_tensor(
                            junk_g,
                            xe[:, pt, 0:D].bitcast(f32),
                            xe[:, pt, 0:D].bitcast(f32),
                            op=ALU.mult,
                        )
                        nc.vector.tensor_scalar(
                            junk_g,
                            junk_g,
                            1.0,
                            0.0,
                            op0=ALU.mult,
                            op1=ALU.add,
                            accum_out=ssq[:, m * NPT + pt : m * NPT + pt + 1],
                        )
                    xes.append(xe)

                # ---------------- phase B: batch stats ----------------
                mu_ap = ytb[:, :, :, 8:9]
                m2 = st_pool.tile([128, NW], f32, tag="m2")
                nc.vector.tensor_mul(m2, mu_ap, mu_ap)
                var = st_pool.tile([128, NW], f32, tag="var")
                nc.vector.scalar_tensor_tensor(
                    var, ssq, 1.0 / D, m2, op0=ALU.mult, op1=ALU.subtract
                )
                # r = rsqrt(var+eps) via exp(-0.5*ln(.)): sqrt's ACT table set
                # lacks exp; batching ln+exp per batch avoids table thrash.
                lnv = st_pool.tile([128, NW], f32, tag="lnv")
                nc.scalar.activation(lnv, var, AF.Ln, bias=eps_t[:, :])
                r_all = st_pool.tile([128, NW], f32, tag="r")
                nc.scalar.activation(r_all, lnv, AF.Exp, scale=-0.5)
                sg_all = st_pool.tile([128, NW], f32, tag="sg")
                nc.vector.reciprocal(sg_all, r_all)

                # ---------------- phase C: u and P1 ----------------
                for m, xe in enumerate(xes):
                    nc.vector.tensor_copy(
                        xe[:, :, D + 1 : D + 2],
                        sg_all[:, m * NPT : (m + 1) * NPT],
                    )

                r_bc = (
                    r_all[:]
                    .rearrange("p (m q) -> p m q", q=NPT)
                    .unsqueeze(3)
                    .to_broadcast([128, nmac, NPT, 8])
                )
                mu_bc = mu_ap.to_broadcast([128, nmac, NPT, 8])
                prod = uw_pool.tile([128, nmac, NPT, 8], f32, tag="prod")
                nc.vector.tensor_mul(prod, mu_bc, s_bc)
                diff = uw_pool.tile([128, nmac, NPT, 8], f32, tag="diff")
                nc.vector.tensor_sub(diff, ytb[:, :, :, 0:8], prod)
                arg = uw_pool.tile([128, nmac, NPT, 8], f32, tag="arg")
                nc.vector.tensor_mul(arg, diff, r_bc)
                if use_c:
                    arg2 = uw_pool.tile([128, nmac, NPT, 8], f32, tag="arg2")
                    nc.vector.tensor_add(arg2, arg, c_bc)
                    arg = arg2
                w_t = uw_pool.tile([128, nmac, NPT, 8], f32, tag="w")
                nc.scalar.activation(w_t, arg, AF.Exp)
                u_all = uw_pool.tile([128, nmac, NPT, 8], f32r, tag="u")
                nc.vector.tensor_mul(u_all, w_t, r_bc)

                p1 = p1p_pool.tile([8, OUTW], f32)
                for m, xe in enumerate(xes):
                    for pt in range(NPT):
                        first = m == 0 and pt == 0
                        last = m == nmac - 1 and pt == NPT - 1
                        nc.tensor.matmul(
                            p1[:, 0:512],
                            u_all[:, m, pt, :],
                            xe[:, pt, 0:512],
                            start=first,
                            stop=last,
                        )
                        nc.tensor.matmul(
                            p1[:, 512:OUTW],
                            u_all[:, m, pt, :],
                            xe[:, pt, 512:OUTW],
                            start=first,
                            stop=last,
                        )

                p1s = osb_pool.tile([8, OUTW], f32)
                nc.vector.tensor_copy(p1s, p1)
                nc.sync.dma_start(out=p1_out[b], in_=p1s)

    return nc


_CACHE = {}


def _get_compiled(bpc, nmac, use_c):
    key = (bpc, nmac, use_c)
    if key not in _CACHE:
        nc = _build(bpc, nmac, use_c)
        nc.compile()
        _CACHE[key] = nc
    return _CACHE[key]


def _round_f32r(a):
    """Round fp32 values to the fp32r grid (11-bit mantissa, RNE) so the PE's
    in-stream truncation is exact."""
    a = np.ascontiguousarray(a, np.float32)
    u = a.view(np.uint32)
    out = (u + np.uint32(0x7FF) + ((u >> np.uint32(12)) & np.uint32(1))) & np.uint32(
        0xFFFFF000
    )
    return out.view(np.float32)


def _host_params(w_kv, query, gamma1, beta1):
    scale = Dh**-0.5
    wk = w_kv[:D]
    qw = (query.reshape(H, Dh)[:, :, None] * wk.reshape(H, Dh, D)).sum(1) * scale
    a = gamma1[None, :] * qw                    # [H, D]
    s = a.sum(-1).astype(np.float32)            # [H]
    c = (beta1[None, :] * qw).sum(-1).astype(np.float32)

    g = np.zeros((D, 9), np.float32)
    g[:, :8] = a.T
    g[:, 8] = 1.0 / D
    g = _round_f32r(g)
    scv = np.zeros((128, 16), np.float32)
    scv[:, 0:8] = s[None, :]
    scv[:, 8:16] = c[None, :]
    ident = np.eye(128, dtype=np.float32)
    return g, scv, ident, c


def kernel(x, w_kv, query, gamma1, beta1, gamma2, beta2, _run_opts=None):
    x = np.asarray(x, np.float32)
    w_kv = np.asarray(w_kv, np.float32)
    query = np.asarray(query, np.float32)
    gamma1 = np.asarray(gamma1, np.float32)
    beta1 = np.asarray(beta1, np.float32)
    gamma2 = np.asarray(gamma2, np.float32)
    beta2 = np.asarray(beta2, np.float32)

    g, scv, ident, c = _host_params(w_kv, query, gamma1, beta1)
    use_c = not np.allclose(c, 0.0)
    nc = _get_compiled(BPC, L // MACRO, use_c)
    xr = _round_f32r(x)
    in_maps = [
        {"x": xr[i * BPC : (i + 1) * BPC], "gmat": g, "scvec": scv, "ident": ident}
        for i in range(NCORES)
    ]
    res = bass_utils.run_bass_kernel_spmd(
        nc, in_maps, core_ids=list(range(NCORES)), **(_run_opts or {})
    )
    p1 = np.concatenate([res.results[i]["p1out"] for i in range(NCORES)], axis=0)

    out = _epilogue(p1, w_kv, gamma1, beta1, gamma2, beta2)
    if _run_opts:
        return out, res
    return out


def _epilogue(p1, w_kv, gamma1, beta1, gamma2, beta2):
    """pooled -> v-projection -> final LayerNorm, on [32,8,768]-sized data."""
    P1 = p1[:, :, :D]
    U = p1[:, :, D]
    Z = p1[:, :, D + 1]
    pooled = gamma1[None, None, :] * (P1 - U[:, :, None]) / Z[:, :, None]
    pooled += beta1[None, None, :]
    wv = w_kv[D:].reshape(H, Dh, D)
    out0 = np.einsum("bhd,hjd->bhj", pooled, wv, optimize=True).reshape(B, D)
    mu = out0.mean(-1, keepdims=True)
    var = out0.var(-1, keepdims=True)
    out = (out0 - mu) / np.sqrt(var + EPS) * gamma2[None, :] + beta2[None, :]
    return out.astype(np.float32)


# revision 8
# speedup vs baseline: 1.2703x; 1.2703x over previous
"""AttentiveReduce Trainium2 kernel.

Reference computation (B=32, L=4096, D=768, H=8, Dh=96):
    xn   = LayerNorm(x; gamma1, beta1)            [B,L,D]
    kv   = xn @ w_kv.T ; k, v = split(kv)         [B,L,D] each
    dots = einsum('hd,blhd->bhl', q, k) * Dh^-0.5
    attn = softmax(dots, axis=-1)
    out  = einsum('bhl,blhd->bhd', attn, v) -> [B,D]
    out  = LayerNorm(out; gamma2, beta2)

Algebraic restructuring (exact up to fp rounding):
  - k only appears via q.k per head, so fold q into Wk on the host:
        qw[h,d] = Dh^-0.5 * sum_j q[h,j] * Wk[h*Dh+j, d]
        dots[b,h,l] = r_l*(x_l . (gamma1*qw_h)) - r_l*mu_l*s_h + c_h
    with LayerNorm stats mu_l, r_l = rsqrt(var_l+eps) and host scalars
    s_h = sum_d gamma1*qw_h, c_h = sum_d beta1*qw_h.
  - v is linear in xn, so pool x first and project after:
        P1[b,h,d] = sum_l u[b,h,l] x[b,l,d],  u = exp(dots)*r_l
        U[b,h] = sum_l u*mu_l,  Z[b,h] = sum_l u*sigma_l  (sigma=1/r)
        pooled = gamma1*(P1 - U)/Z + beta1 ; out = pooled @ Wv_h.T ; LN2
  - |dots| stays ~5 for this data, so softmax needs no max subtraction:
    one streaming pass over x.

Device computes P1/U/Z; the tiny epilogue runs on host over [32,8,768].

Device pipeline per batch (L=4096 = 8 macro tiles x 512 tokens):
  phase A (per macro): DMA x; PE-transpose x into d-partition layout;
    f32r logit matmul Y = [a_0..a_7, ones/D]^T @ x^T; transpose Y back to
    token-partition layout; per-token sum(x^2) split between ACT
    (Square+accum) and GPSIMD(mult)+DVE(accum).
  phase B (per batch): var -> r = exp(-0.5*ln(var+eps)) -> sigma, batched
    [128, 32] so the ACT exp/ln table sets load only twice per batch.
  phase C: batched u = exp(r*(y - mu*s) + c)*r over the whole batch, then
    per-p-tile f32r matmuls accumulate P1 = u^T @ [x | mu | sigma] in PSUM.

Sharding: data-parallel over batch: 8 cores x 4 batches, params replicated.
Per-core HBM traffic = 48MiB of x read once (memory-bound target).
fp32r (fp32 with 11-bit mantissa, 4x PE throughput) is used for the
matmuls; inputs are pre-rounded on host so HW truncation is exact RNE.
"""

import sys

if "/opt/trn_rl_repo" not in sys.path:
    sys.path.insert(0, "/opt/trn_rl_repo")

import numpy as np

import concourse.bacc as bacc
import concourse.tile as tile
from concourse import bass_utils, mybir

f32 = mybir.dt.float32
f32r = mybir.dt.float32r
AF = mybir.ActivationFunctionType
ALU = mybir.AluOpType

B, L, D, H, Dh = 32, 4096, 768, 8, 96
EPS = 1e-5
NCORES = 8
BPC = B // NCORES  # batches per core
PT = 128           # tokens per partition tile
MACRO = 512        # tokens per macro tile (4 p-tiles)
NPT = MACRO // PT  # 4
NC6 = D // 128     # 6 d-chunks of 128
OUTW = D + 2       # P1 row width: 768 x-cols + mu col + sigma col


def _build(bpc, nmac, use_c):
    """Per-core program: `bpc` batches x nmac*512 tokens each."""
    nc = bacc.Bacc("TRN2", target_bir_lowering=False, debug=False)

    x_in = nc.dram_tensor("x", [bpc, nmac * MACRO, D], f32r, kind="ExternalInput")
    g_in = nc.dram_tensor("gmat", [D, 9], f32r, kind="ExternalInput")
    sc_in = nc.dram_tensor("scvec", [128, 16], f32, kind="ExternalInput")
    id_in = nc.dram_tensor("ident", [128, 128], f32r, kind="ExternalInput")
    p1_out = nc.dram_tensor("p1out", [bpc, 8, OUTW], f32, kind="ExternalOutput")

    NW = nmac * NPT  # p-tiles per batch (32)

    with tile.TileContext(nc) as tc:
        with (
            tc.tile_pool(name="singles", bufs=1) as singles,
            tc.tile_pool(name="xe", bufs=nmac + 3) as xe_pool,
            tc.tile_pool(name="xt", bufs=2) as xt_pool,
            tc.tile_pool(name="ysb", bufs=2) as ysb_pool,
            tc.tile_pool(name="ytb", bufs=2) as ytb_pool,
            tc.tile_pool(name="uw", bufs=2) as uw_pool,
            tc.tile_pool(name="st", bufs=2) as st_pool,
            tc.tile_pool(name="junk", bufs=1) as junk_pool,
            tc.tile_pool(name="osb", bufs=2) as osb_pool,
            tc.tile_pool(name="ptp", bufs=4, space="PSUM") as ptp_pool,
            tc.tile_pool(name="yp", bufs=1, space="PSUM") as yp_pool,
            tc.tile_pool(name="ytp", bufs=1, space="PSUM") as ytp_pool,
            tc.tile_pool(name="p1p", bufs=1, space="PSUM") as p1p_pool,
        ):
            g_sb = singles.tile([128, NC6, 9], f32r)
            nc.sync.dma_start(out=g_sb, in_=g_in.rearrange("(c p) m -> p c m", p=128))
            sc_sb = singles.tile([128, 16], f32)
            nc.sync.dma_start(out=sc_sb, in_=sc_in[:, :])
            id_sb = singles.tile([128, 128], f32r)
            nc.sync.dma_start(out=id_sb, in_=id_in[:, :])
            eps_t = singles.tile([128, 1], f32)
            nc.vector.memset(eps_t, EPS)

            s_bc = (
                sc_sb[:, 0:8]
                .unsqueeze(1)
                .unsqueeze(1)
                .to_broadcast([128, nmac, NPT, 8])
            )
            c_bc = (
                sc_sb[:, 8:16]
                .unsqueeze(1)
                .unsqueeze(1)
                .to_broadcast([128, nmac, NPT, 8])
            )

            for b in range(bpc):
                # ---------------- phase A: stream macros ----------------
                xes = []
                ytb = ytb_pool.tile([128, nmac, NPT, 9], f32)
                ssq = st_pool.tile([128, NW], f32, tag="ssq")
                for m in range(nmac):
                    xe = xe_pool.tile([128, NPT, OUTW], f32r)
                    for hh in range(2):
                        src = x_in[
                            b, m * MACRO + hh * 256 : m * MACRO + (hh + 1) * 256, :
                        ].rearrange("(pt p) d -> p pt d", p=128)
                        nc.sync.dma_start(
                            out=xe[:, 2 * hh : 2 * hh + 2, 0:D], in_=src
                        )

                    xt = xt_pool.tile([128, NC6, MACRO], f32r)
                    for c in range(NC6):
                        xtp = ptp_pool.tile([128, MACRO], f32)
                        for pt in range(NPT):
                            nc.tensor.transpose(
                                xtp[:, pt * PT : (pt + 1) * PT].bitcast(f32r),
                                xe[:, pt, c * 128 : (c + 1) * 128],
                                id_sb[:, :],
                            )
                        nc.vector.tensor_copy(xt[:, c, :], xtp)

                    # Y rows 0-7 = x . a_h, row 8 = mu
                    yp = yp_pool.tile([9, MACRO], f32)
                    for c in range(NC6):
                        nc.tensor.matmul(
                            yp,
                            g_sb[:, c, :],
                            xt[:, c, :],
                            start=(c == 0),
                            stop=(c == NC6 - 1),
                        )
                    y_sb = ysb_pool.tile([9, MACRO], f32)
                    nc.vector.tensor_copy(y_sb, yp)
                    ytp = ytp_pool.tile([128, NPT, 9], f32)
                    for pt in range(NPT):
                        nc.tensor.transpose(
                            ytp[:, pt, :],
                            y_sb[:, pt * PT : (pt + 1) * PT],
                            id_sb[:9, :9].bitcast(f32),
                        )
                    nc.vector.tensor_copy(ytb[:, m, :, :], ytp)
                    # mu into the U column of x_ext
                    nc.vector.tensor_copy(xe[:, :, D : D + 1], ytp[:, :, 8:9])

                    # per-token sum(x^2) on ACT (Square + free-dim accum)
                    junk_a = junk_pool.tile([128, D], f32, tag="junk_a")
                    for pt in range(NPT):
                        nc.scalar.activation(
                            junk_a,
                            xe[:, pt, 0:D].bitcast(f32),
                            AF.Square,
                            accum_out=ssq[:, m * NPT + pt : m * NPT + pt + 1],
                        )
                    xes.append(xe)

                # ---------------- phase B: batch stats ----------------
                mu_ap = ytb[:, :, :, 8:9]
                m2 = st_pool.tile([128, NW], f32, tag="m2")
                nc.vector.tensor_mul(m2, mu_ap, mu_ap)
                var = st_pool.tile([128, NW], f32, tag="var")
                nc.vector.scalar_tensor_tensor(
                    var, ssq, 1.0 / D, m2, op0=ALU.mult, op1=ALU.subtract
                )
                # r = rsqrt(var+eps) via exp(-0.5*ln(.)): sqrt's ACT table set
                # lacks exp; batching ln+exp per batch avoids table thrash.
                lnv = st_pool.tile([128, NW], f32, tag="lnv")
                nc.scalar.activation(lnv, var, AF.Ln, bias=eps_t[:, :])
                r_all = st_pool.tile([128, NW], f32, tag="r")
                nc.scalar.activation(r_all, lnv, AF.Exp, scale=-0.5)
                sg_all = st_pool.tile([128, NW], f32, tag="sg")
                nc.vector.reciprocal(sg_all, r_all)

                # ---------------- phase C: u and P1 ----------------
                for m, xe in enumerate(xes):
                    nc.vector.tensor_copy(
                        xe[:, :, D + 1 : D + 2],
                        sg_all[:, m * NPT : (m + 1) * NPT],
                    )

                r_bc = (
                    r_all[:]
                    .rearrange("p (m q) -> p m q", q=NPT)
                    .unsqueeze(3)
                    .to_broadcast([128, nmac, NPT, 8])
                )
                mu_bc = mu_ap.to_broadcast([128, nmac, NPT, 8])
                prod = uw_pool.tile([128, nmac, NPT, 8], f32, tag="prod")
                nc.vector.tensor_mul(prod, mu_bc, s_bc)
                diff = uw_pool.tile([128, nmac, NPT, 8], f32, tag="diff")
                nc.vector.tensor_sub(diff, ytb[:, :, :, 0:8], prod)
                arg = uw_pool.tile([128, nmac, NPT, 8], f32, tag="arg")
                nc.vector.tensor_mul(arg, diff, r_bc)
                if use_c:
                    arg2 = uw_pool.tile([128, nmac, NPT, 8], f32, tag="arg2")
                    nc.vector.tensor_add(arg2, arg, c_bc)
                    arg = arg2
                w_t = uw_pool.tile([128, nmac, NPT, 8], f32, tag="w")
                nc.scalar.activation(w_t, arg, AF.Exp)
                u_all = uw_pool.tile([128, nmac, NPT, 8], f32r, tag="u")
                nc.vector.tensor_mul(u_all, w_t, r_bc)

                p1 = p1p_pool.tile([8, OUTW], f32)
                for m, xe in enumerate(xes):
                    for pt in range(NPT):
                        first = m == 0 and pt == 0
                        last = m == nmac - 1 and pt == NPT - 1
                        nc.tensor.matmul(
                            p1[:, 0:512],
                            u_all[:, m, pt, :],
                            xe[:, pt, 0:512],
                            start=first,
                            stop=last,
                        )
                        nc.tensor.matmul(
                            p1[:, 512:OUTW],
                            u_all[:, m, pt, :],
                            xe[:, pt, 512:OUTW],
                            start=first,
                            stop=last,
                        )

                p1s = osb_pool.tile([8, OUTW], f32)
                nc.vector.tensor_copy(p1s, p1)
                nc.sync.dma_start(out=p1_out[b], in_=p1s)

    return nc


_CACHE = {}


def _get_compiled(bpc, nmac, use_c):
    key = (bpc, nmac, use_c)
    if key not in _CACHE:
        nc = _build(bpc, nmac, use_c)
        nc.compile()
        _CACHE[key] = nc
    return _CACHE[key]


def _round_f32r(a):
    """Round fp32 values to the fp32r grid (11-bit mantissa, RNE) so the PE's
    in-stream truncation is exact."""
    a = np.ascontiguousarray(a, np.float32)
    u = a.view(np.uint32)
    out = (u + np.uint32(0x7FF) + ((u >> np.uint32(12)) & np.uint32(1))) & np.uint32(
        0xFFFFF000
    )
    return out.view(np.float32)


def _host_params(w_kv, query, gamma1, beta1):
    scale = Dh**-0.5
    wk = w_kv[:D]
    qw = (query.reshape(H, Dh)[:, :, None] * wk.reshape(H, Dh, D)).sum(1) * scale
    a = gamma1[None, :] * qw                    # [H, D]
    s = a.sum(-1).astype(np.float32)            # [H]
    c = (beta1[None, :] * qw).sum(-1).astype(np.float32)

    g = np.zeros((D, 9), np.float32)
    g[:, :8] = a.T
    g[:, 8] = 1.0 / D
    g = _round_f32r(g)
    scv = np.zeros((128, 16), np.float32)
    scv[:, 0:8] = s[None, :]
    scv[:, 8:16] = c[None, :]
    ident = np.eye(128, dtype=np.float32)
    return g, scv, ident, c


def kernel(x, w_kv, query, gamma1, beta1, gamma2, beta2, _run_opts=None):
    x = np.asarray(x, np.float32)
    w_kv = np.asarray(w_kv, np.float32)
    query = np.asarray(query, np.float32)
    gamma1 = np.asarray(gamma1, np.float32)
    beta1 = np.asarray(beta1, np.float32)
    gamma2 = np.asarray(gamma2, np.float32)
    beta2 = np.asarray(beta2, np.float32)

    g, scv, ident, c = _host_params(w_kv, query, gamma1, beta1)
    use_c = not np.allclose(c, 0.0)
    nc = _get_compiled(BPC, L // MACRO, use_c)
    xr = _round_f32r(x)
    in_maps = [
        {"x": xr[i * BPC : (i + 1) * BPC], "gmat": g, "scvec": scv, "ident": ident}
        for i in range(NCORES)
    ]
    res = bass_utils.run_bass_kernel_spmd(
        nc, in_maps, core_ids=list(range(NCORES)), **(_run_opts or {})
    )
    p1 = np.concatenate([res.results[i]["p1out"] for i in range(NCORES)], axis=0)

    out = _epilogue(p1, w_kv, gamma1, beta1, gamma2, beta2)
    if _run_opts:
        return out, res
    return out


def _epilogue(p1, w_kv, gamma1, beta1, gamma2, beta2):
    """pooled -> v-projection -> final LayerNorm, on [32,8,768]-sized data."""
    P1 = p1[:, :, :D]
    U = p1[:, :, D]
    Z = p1[:, :, D + 1]
    pooled = gamma1[None, None, :] * (P1 - U[:, :, None]) / Z[:, :, None]
    pooled += beta1[None, None, :]
    wv = w_kv[D:].reshape(H, Dh, D)
    out0 = np.einsum("bhd,hjd->bhj", pooled, wv, optimize=True).reshape(B, D)
    mu = out0.mean(-1, keepdims=True)
    var = out0.var(-1, keepdims=True)
    out = (out0 - mu) / np.sqrt(var + EPS) * gamma2[None, :] + beta2[None, :]
    return out.astype(np.float32)
